# revision 49
# baseline (speedup 1.0000x reference)
"""Causal self-attention with RoPE on 8 TRN2 NeuronCores.

Sharding: core c -> (batch b = c//4, head-group g = c%4; 4 heads of 128 each).
Tensor-parallel over heads x data-parallel over batch.

Single fused pipeline per core, one pass over x, token chunks of
[512, 512, 512, 256, 256]. Per chunk: QKV matmuls (+RoPE on Q,K) -> causal
attention over kv-blocks <= chunk end -> project own heads through own W_proj
row-block into a full-D partial z -> per-chunk ReduceScatter(add) within the
batch group produces this core's final 512-dim slice of z^T.

The projection for chunk i is EMITTED inside chunk i+1 (after its QKV
matmuls): the softmax-normalize latency chain (rowsum -> reciprocal ->
broadcast -> scale) then hides under 40us of next-chunk PE work instead of
stalling the in-order PE at every chunk boundary. The two final 256-token
chunks shrink the only exposed collective (last ReduceScatter: 21.5us vs
28us) and the tail projection.

All matmul operands are fp16 (1 PE cycle/row, same as bf16, 10-bit
mantissa). PSUM accumulation and softmax statistics stay fp32.

DMA queues: sync = input prefetch in consumption order + RoPE rotate-half
swaps; Act = z-partial spills (their producer is the Act PSUM-drain copy);
Pool = collectives + 1/rowsum partition_broadcast. Big input transfers are
batched (2 DMAs per x chunk / weight matrix) but halved so the first
accumulation chain can start after ~6us.

Layouts (no on-chip transposes needed):
  xT   [D, S]   = x[b].T                      (host-transposed)
  Q^T,K^T [128, S] per head  (from matmul: lhsT=W-block, rhs=xT)
  V    [S, 512] token-major  (from matmul: lhsT=xT-tile, rhs=Wv)
  S^T  [j, i] scores blocks -> softmax sums via ones-matmul on PE
  O^T  [c, i] accumulated in PSUM, normalized by 1/rowsum afterwards
  zp   [D, cw] partial projection, ReduceScatter -> z_red [512, cw] -> zT
"""
from contextlib import ExitStack

import numpy as np

import concourse.bass as bass
import concourse.tile as tile
import concourse.mybir as mybir
from concourse import bacc, bass_utils

B = 2
S = 2048
D = 2048
NH, HD = 16, 128
HPC = 4                 # heads per core
EL = HPC * HD           # 512: local e-width per core
CH = 512                # max token-chunk width
CHUNKS = [(0, 512), (512, 512), (1024, 512), (1536, 384), (1920, 128)]
DT = D // 128           # 16 d-tiles
KT = EL // 128          # 4 k-tiles of the local proj contraction
ROPE_THETA = 10000.0
N_CORES = 8
GROUPS = [[0, 1, 2, 3], [4, 5, 6, 7]]

F32 = mybir.dt.float32
F16 = mybir.dt.float16
AF = mybir.ActivationFunctionType


def _build():
    nc = bacc.Bacc("TRN2", target_bir_lowering=False, debug=False,
                   enable_asserts=True, num_devices=N_CORES)
    xT = nc.dram_tensor("xT", [D, S], F16, kind="ExternalInput").ap()
    # wq/wk arrive host pre-tiled: [128 partitions, (d-tile, e)] so any
    # column range is a contiguous >=4KB run per partition
    wq = nc.dram_tensor("wq", [128, DT * EL], F16, kind="ExternalInput").ap()
    wk = nc.dram_tensor("wk", [128, DT * EL], F16, kind="ExternalInput").ap()
    wv = nc.dram_tensor("wv", [D, EL], F16, kind="ExternalInput").ap()
    wp = nc.dram_tensor("wp", [EL, D], F16, kind="ExternalInput").ap()
    cosq = nc.dram_tensor("cosq", [HD, S], F16, kind="ExternalInput").ap()
    sinq = nc.dram_tensor("sinq", [HD, S], F16, kind="ExternalInput").ap()
    cosk = nc.dram_tensor("cosk", [HD, S], F16, kind="ExternalInput").ap()
    sink = nc.dram_tensor("sink", [HD, S], F16, kind="ExternalInput").ap()
    tri = nc.dram_tensor("tri", [128, 128], F16, kind="ExternalInput").ap()
    ident = nc.dram_tensor("ident", [128, 128], F16, kind="ExternalInput").ap()
    zc = nc.dram_tensor("zc", [EL, S], F16, kind="ExternalOutput").ap()

    # [p, t, e] views: 128 partitions, d-tiles stacked along a middle dim
    wvv = wv.rearrange("(t p) e -> p t e", p=128)
    wpv = wp.rearrange("(k p) d -> p k d", p=128)
    xTv = xT.rearrange("(t p) s -> p t s", p=128)

    HW = 8 * CH          # half-width of a batched x / qkv-weight transfer

    with tile.TileContext(nc) as tc, \
         nc.allow_low_precision(reason="fp16 attention"), ExitStack() as ctx:
        cpool = ctx.enter_context(tc.tile_pool(name="const", bufs=1))
        wpool = ctx.enter_context(tc.tile_pool(name="wts", bufs=1))
        xpool = ctx.enter_context(tc.tile_pool(name="xc", bufs=2))
        kres = ctx.enter_context(tc.tile_pool(name="kres", bufs=4))
        vres = ctx.enter_context(tc.tile_pool(name="vres", bufs=64))
        qpool = ctx.enter_context(tc.tile_pool(name="qp", bufs=5))
        prepool = ctx.enter_context(tc.tile_pool(name="pre", bufs=8))
        rpool = ctx.enter_context(tc.tile_pool(name="rope", bufs=2))
        ppool = ctx.enter_context(tc.tile_pool(name="pp", bufs=4))
        opool = ctx.enter_context(tc.tile_pool(name="op", bufs=4))
        spool = ctx.enter_context(tc.tile_pool(name="sm", bufs=3))
        ypool = ctx.enter_context(tc.tile_pool(name="yp", bufs=9))
        zpool = ctx.enter_context(tc.tile_pool(name="zp", bufs=2))
        dram = ctx.enter_context(tc.tile_pool(name="dram", bufs=1, space="DRAM"))
        ps_mm = ctx.enter_context(tc.tile_pool(name="ps_mm", bufs=2, space="PSUM"))
        ps_sc = ctx.enter_context(tc.tile_pool(name="ps_sc", bufs=2, space="PSUM"))
        ps_o = ctx.enter_context(tc.tile_pool(name="ps_o", bufs=4, space="PSUM"))

        # ---- prefetch stream (sync queue order == arrival priority) ----
        tri_t = cpool.tile([128, 128], F16)
        nc.sync.dma_start(tri_t[:], tri)
        ident_t = cpool.tile([128, 128], F16)
        nc.sync.dma_start(ident_t[:], ident)

        QW = 4 * EL          # quarter-width of a wk/wq transfer (4 d-tiles)
        wk_p = [wpool.tile([128, QW], F16, name=f"wk_p{i}") for i in range(4)]
        wq_p = [wpool.tile([128, QW], F16, name=f"wq_p{i}") for i in range(4)]
        wv_t = wpool.tile([128, 2 * HW], F16, name="wv")
        wp_t = wpool.tile([128, KT * D], F16, name="wp")

        def xw_slice(parts, dt, cw, c0, c1):
            base = (dt % 4) * cw
            return parts[dt // 4][:, base + c0:base + c1]

        xc_t = {}

        def x_load(ci, interleave=None):
            # 4 DMAs per chunk, each covering 4 d-tiles x cw tokens
            start, cw = CHUNKS[ci]
            parts = []
            for i in range(4):
                if interleave is not None:
                    interleave(i)
                xp = xpool.tile([128, 4 * cw], F16, tag=f"x{i}",
                                name=f"x{ci}_{i}")
                nc.sync.dma_start(
                    xp[:].rearrange("p (t c) -> p t c", t=4),
                    xTv[:, 4 * i:4 * i + 4, start:start + cw])
                parts.append(xp)
            xc_t[ci] = parts

        # interleave wk quarters with x0 quarters: the first K chain starts
        # after ~2 small transfers instead of the full 4.2MB
        x_load(0, interleave=lambda i: nc.sync.dma_start(
            wk_p[i][:], wk[:, i * QW:(i + 1) * QW]))
        for i in range(4):
            nc.sync.dma_start(wq_p[i][:], wq[:, i * QW:(i + 1) * QW])
        nc.sync.dma_start(
            wv_t[:].rearrange("p (t c) -> p t c", t=16), wvv)
        csk = cpool.tile([HD, S], F16, name="cosk")
        nc.sync.dma_start(csk[:], cosk)
        snk = cpool.tile([HD, S], F16, name="sink")
        nc.sync.dma_start(snk[:], sink)
        csq = cpool.tile([HD, S], F16, name="cosq")
        nc.sync.dma_start(csq[:], cosq)
        snq = cpool.tile([HD, S], F16, name="sinq")
        nc.sync.dma_start(snq[:], sinq)

        k_t = [kres.tile([HD, S], F16, tag="k", name=f"k{h}")
               for h in range(HPC)]
        # per-(token-block, head) V tiles with a trailing ones column: the
        # token-major PV matmul then accumulates the softmax row-sum in
        # output column HD for free (no separate ones-matmul pass)
        v_t = [vres.tile([128, HD + 1], F16, tag="v", name=f"v{i}")
               for i in range((S // 128) * HPC)]
        for vt in v_t:
            nc.vector.memset(vt[:, HD:HD + 1], 1.0)
        z_part = [dram.tile([D, cw], F16, tag=f"zp{ci}", name=f"zp{ci}")
                  for ci, (_, cw) in enumerate(CHUNKS)]
        z_red = [dram.tile([EL, cw], F16, tag=f"zr{ci}", name=f"zr{ci}")
                 for ci, (_, cw) in enumerate(CHUNKS)]

        def rope_pre(ps, cw):
            """Phase A: drain the QKV PSUM chain to SBUF (frees the bank so
            the next PE chain never waits on downstream RoPE progress)."""
            pre = prepool.tile([128, cw], F16, tag="pre")
            nc.scalar.copy(pre[:], ps[:])
            return pre

        def rope_apply(pre, cw, cs, sn, out_tile, sl):
            """Phase B: out = pre*cos + rotate_half(pre)*sin (sign in sin)."""
            rot = rpool.tile([128, cw], F16, tag="rot")
            nc.sync.dma_start(rot[0:64, :], pre[64:128, :])
            nc.sync.dma_start(rot[64:128, :], pre[0:64, :])
            t1 = rpool.tile([128, cw], F16, tag="t1")
            nc.vector.tensor_mul(t1[:], pre[:], cs)
            t2 = rpool.tile([128, cw], F16, tag="t2")
            nc.vector.tensor_mul(t2[:], rot[:], sn)
            nc.vector.tensor_add(out_tile[:, sl], t1[:], t2[:])

        def proj_chunk(ci, y_sb):
            """Partial projection zp[ci] = Wp[g-rows]^T @ y, spill, and
            ReduceScatter. Emitted one chunk late (see module docstring)."""
            cw = CHUNKS[ci][1]
            for half in range(2):
                zb = zpool.tile([128, 8 * cw], F16, tag="zb")
                for dd in range(8):
                    d2 = 8 * half + dd
                    ps = ps_mm.tile([128, cw], F32)
                    for kk in range(KT):
                        nc.tensor.matmul(
                            ps[:],
                            wp_t[:, kk * D + d2 * 128:kk * D + (d2 + 1) * 128],
                            y_sb[kk][:], start=(kk == 0), stop=(kk == KT - 1))
                    nc.scalar.copy(zb[:, dd * cw:(dd + 1) * cw], ps[:])
                dst = z_part[ci][half * 8 * 128:(half + 1) * 8 * 128, :]
                nc.scalar.dma_start(
                    dst.rearrange("(t p) c -> p t c", p=128),
                    zb[:].rearrange("p (t c) -> p t c", t=8))
            nc.gpsimd.collective_compute(
                "ReduceScatter", mybir.AluOpType.add,
                replica_groups=GROUPS,
                ins=[z_part[ci].opt()], outs=[z_red[ci].opt()])

        pending_proj = None
        for ci, (start, cw) in enumerate(CHUNKS):
            tsl = slice(start, start + cw)
            if ci + 1 < len(CHUNKS):
                # prefetch next chunk's x ahead of the bulkier wp load so the
                # in-order DMA queue matches consumption order
                x_load(ci + 1)
            if ci == 0:
                nc.sync.dma_start(
                    wp_t[:].rearrange("p (k c) -> p k c", k=KT), wpv)
            xparts = xc_t[ci]

            # ---------------- K chunk (+RoPE) ----------------
            k_pre = []
            for h in range(HPC):
                ps = ps_mm.tile([HD, cw], F32)
                for dt in range(DT):
                    nc.tensor.matmul(
                        ps[:],
                        xw_slice(wk_p, dt, EL, h * HD, (h + 1) * HD),
                        xw_slice(xparts, dt, cw, 0, cw),
                        start=(dt == 0), stop=(dt == DT - 1))
                k_pre.append(rope_pre(ps, cw))

            # ---------------- Q chunk (+RoPE) ----------------
            qc = []
            q_pre = []
            for h in range(HPC):
                ps = ps_mm.tile([HD, cw], F32)
                for dt in range(DT):
                    nc.tensor.matmul(
                        ps[:],
                        xw_slice(wq_p, dt, EL, h * HD, (h + 1) * HD),
                        xw_slice(xparts, dt, cw, 0, cw),
                        start=(dt == 0), stop=(dt == DT - 1))
                q_pre.append(rope_pre(ps, cw))
                qc.append(qpool.tile([HD, cw], F16, tag="q", name=f"q{ci}_{h}"))
            for h in range(HPC):
                rope_apply(k_pre[h], cw, csk[:, tsl], snk[:, tsl], k_t[h], tsl)
                rope_apply(q_pre[h], cw, csq[:, tsl], snq[:, tsl], qc[h],
                           slice(None))

            # ---------------- V chunk (token-major) ----------------
            for st in range(cw // 128):
                ps = ps_mm.tile([128, EL], F32)
                for dt in range(DT):
                    nc.tensor.matmul(
                        ps[:],
                        xw_slice(xparts, dt, cw, st * 128, (st + 1) * 128),
                        wv_t[:, dt * EL:(dt + 1) * EL],
                        start=(dt == 0), stop=(dt == DT - 1))
                for h in range(HPC):
                    nc.vector.tensor_copy(
                        v_t[(start // 128 + st) * HPC + h][:, 0:HD],
                        ps[:, h * HD:(h + 1) * HD])

            # previous chunk's projection: fills the PE while this chunk's
            # RoPE completes, and hides the previous normalize latency
            if pending_proj is not None:
                proj_chunk(*pending_proj)

            # ---------------- causal attention for this chunk ----------------
            # token-major: per query-block ib, accumulate o[i, 0:HD] = p^T V
            # and the softmax row-sum in o[i, HD] (ones column of V), then
            # normalize per-partition and transpose back to head-major.
            y_sb = []
            n_jt = (start + cw) // 128
            n_ib = cw // 128
            for h in range(HPC):
                o_ps = [ps_o.tile([128, HD + 1], F32, tag="ops",
                                  name=f"ops{ci}_{h}_{ib}")
                        for ib in range(n_ib)]
                for jt in range(n_jt):
                    blk = jt * 128 - start
                    off = blk if blk > 0 else 0
                    s_ps = ps_sc.tile([128, cw], F32)
                    nc.tensor.matmul(
                        s_ps[:, off:], k_t[h][:, jt * 128:(jt + 1) * 128],
                        qc[h][:, off:], start=True, stop=True)
                    p = ppool.tile([128, cw], F16, tag="p")
                    nc.scalar.activation(p[:, off:], s_ps[:, off:], AF.Exp)
                    if blk >= 0:
                        nc.vector.tensor_mul(
                            p[:, off:off + 128], p[:, off:off + 128], tri_t[:])
                    for ib in range(max(0, blk) // 128, n_ib):
                        nc.tensor.matmul(
                            o_ps[ib][:], p[:, ib * 128:(ib + 1) * 128],
                            v_t[jt * HPC + h][:],
                            start=(jt == 0),
                            stop=(jt == start // 128 + ib))
                yt = ypool.tile([HD, cw], F16, tag="y", name=f"y{ci}_{h}")
                for ib in range(n_ib):
                    rinv = spool.tile([128, 1], F32, tag="rinv")
                    nc.vector.reciprocal(rinv[:], o_ps[ib][:, HD:HD + 1])
                    ytok = opool.tile([128, HD], F16, tag="o")
                    nc.vector.tensor_scalar_mul(
                        ytok[:], o_ps[ib][:, 0:HD], rinv[:])
                    tr = ps_mm.tile([128, HD], F16, tag="ps",
                                    name=f"tr{ci}_{h}_{ib}")
                    nc.tensor.transpose(tr[:], ytok[:], ident_t[:])
                    nc.scalar.copy(yt[:, ib * 128:(ib + 1) * 128], tr[:])
                y_sb.append(yt)
            pending_proj = (ci, y_sb)

        proj_chunk(*pending_proj)

        for ci, (start, cw) in enumerate(CHUNKS):
            nc.sync.dma_start(zc[:, start:start + cw], z_red[ci][:])
    nc.compile()
    return nc


def _tables():
    inv_freq = 1.0 / (ROPE_THETA ** (np.arange(0, HD, 2, dtype=np.float64) / HD))
    pos = np.arange(S, dtype=np.float64)
    f_half = np.outer(inv_freq, pos)                  # [64, S]
    freqs = np.concatenate([f_half, f_half], axis=0)  # [HD, S]
    # match reference numerics: cos/sin computed in float32 domain
    emb32 = freqs.astype(np.float32)
    cos_t = np.cos(emb32)
    sin_t = np.sin(emb32)
    scale = np.float32(HD ** -0.5)
    sgn = np.where(np.arange(HD) < HD // 2, -1.0, 1.0).astype(np.float32)[:, None]
    cosq = (cos_t * scale).astype(np.float16)
    sinq = (sin_t * sgn * scale).astype(np.float16)
    cosk = cos_t.astype(np.float16)
    sink = (sin_t * sgn).astype(np.float16)
    return cosq, sinq, cosk, sink


_NC_CACHE = {}


def _get_nc():
    if "nc" not in _NC_CACHE:
        _NC_CACHE["nc"] = _build()
    return _NC_CACHE["nc"]


def make_in_maps(x, W_attn, W_proj):
    x = np.asarray(x, dtype=np.float32)
    W_attn = np.asarray(W_attn, dtype=np.float32)
    W_proj = np.asarray(W_proj, dtype=np.float32)
    cosq, sinq, cosk, sink = _tables()
    tri = np.triu(np.ones((128, 128), np.float16))   # [jj, ii]: keep jj <= ii
    ident = np.eye(128, dtype=np.float16)
    def pretile(w):  # [D, EL] -> [128, (d-tile, e)] partition-major tiling
        return np.ascontiguousarray(
            w.reshape(DT, 128, EL).transpose(1, 0, 2).reshape(128, DT * EL)
        ).astype(np.float16)

    in_maps = []
    for c in range(N_CORES):
        b, g = divmod(c, HPC)
        in_maps.append({
            "xT": np.ascontiguousarray(x[b].T).astype(np.float16),
            "wq": pretile(W_attn[:, g * EL:(g + 1) * EL]),
            "wk": pretile(W_attn[:, D + g * EL:D + (g + 1) * EL]),
            "wv": W_attn[:, 2 * D + g * EL:2 * D + (g + 1) * EL].astype(np.float16),
            "wp": W_proj[g * EL:(g + 1) * EL, :].astype(np.float16),
            "cosq": cosq, "sinq": sinq, "cosk": cosk, "sink": sink,
            "tri": tri, "ident": ident,
        })
    return in_maps


def assemble(results):
    out = np.empty((B, S, D), dtype=np.float32)
    for c in range(N_CORES):
        b, g = divmod(c, HPC)
        zT = np.asarray(results[c]["zc"], dtype=np.float32)
        out[b, :, g * EL:(g + 1) * EL] = zT.T
    return out


def kernel(x, W_attn, W_proj):
    nc = _get_nc()
    in_maps = make_in_maps(x, W_attn, W_proj)
    res = bass_utils.run_bass_kernel_spmd(
        nc, in_maps, core_ids=list(range(N_CORES)), trace=False)
    return assemble(res.results)


if __name__ == "__main__":
    rng = np.random.default_rng(0)
    x = rng.standard_normal((B, S, D)).astype(np.float32)
    W_attn = (rng.standard_normal((D, 3 * D)) * D ** -0.5).astype(np.float32)
    W_proj = (rng.standard_normal((D, D)) * D ** -0.5).astype(np.float32)
    out = kernel(x, W_attn, W_proj)
    print("out", out.shape, out.dtype, np.abs(out).mean())
